# revision 22
# baseline (speedup 1.0000x reference)
"""MinGRU Trainium2 kernel (nn_MinGRU_60421599920446) — V2.

Math (per batch row), z = sigmoid(x@wz^T + bz), vh = x@wh^T:
    h_t = (1-z_t) h_{t-1} + z_t (vh_t + bh)
With c_t = h_t - bh the bh bias drops out on device:
    c_t = (1-z_t) c_{t-1} + z_t vh_t,   c_0 = h0 - bh
The host adds bh back (and transposes) when assembling the output.

Data-parallel over batch, 1 row per NeuronCore (8 cores).

V2 design (vs V1 baseline at ~58us):
  * MINGRU_SCAN3_ANT: interleaved-halves custom DVE scan at 1 slot per
    (t, half) — 2x the V1 rate — seeded with the two per-half carries
    from the s0/s1 scalar operands by a 2-slot seed uop (writes the
    s4/s5 A-flops via BYPASS; 2 slots so the first steady element reads
    a latched value).
  * The v stream is read DIRECTLY from PSUM fp32 (PSUM has exactly one
    DVE read port; z comes from SBUF) — the V1 PSUM->SBUF staging copy
    (2048 cols/chunk of ACT time) is gone entirely.
  * bf16 output (halves the out-DMA bytes; rel-err budget 2e-2 >> bf16).
  * Host packs x per-chunk into [128, k0|k1] slabs -> every DMA moves
    4KB contiguous per partition.
  * Scan runs per 512-step segment so vh PSUM tiles ([128,1024] = 2
    banks, both halves) double-buffer within the 8-bank budget:
    vz 2x2 banks + vh 2x2 banks.
  * Dummy sigmoid at t=0 pre-loads the ACT table off the critical path.

Out layout: out_d[p, 2*T + m] = c[T, m*128 + p]  (interleaved halves).
"""

import numpy as np
from contextlib import ExitStack

B, S, D, H = 8, 8192, 256, 256
N_CORES = 8
SEG = 512

_CACHE = {}

OP3_NAME = "MINGRU_SCAN3_ANT"


def _register_mingru3_op():
    """Interleaved-halves fused scan, carries via s0/s1 scalars.

    Stream slots alternate halves (even slot = half0, odd = half1); the
    call site's 3D APs [p, s, m] deliver (t, m)-interleaved elements.
    Each slot consumes (z, v) of its half and performs the full update
    c = (1-z)c + z*v. Per-half state lives in a stage A-flop (half0:
    s4.A, half1: s5.A), read one stage earlier via NEXT_ALU_OUT_A: a
    2-cycle feedback loop matching each half's 2-slot cadence.

    A 2-slot seed uop plants s0 -> s4.A and s1 -> s5.A through BYPASS
    ALUs (consumes nothing, emits nothing). Two slots guarantee the
    first steady element (reading s4.A at entry+3) sees the value the
    first seed slot latched at entry+2 even if the flop read is
    pre-latch. BYPASS seeding also kills any NaN junk left in the
    A-flops by earlier work."""
    import concourse.dve_ops as dve_ops
    for o in dve_ops.OPS:
        if o.name == OP3_NAME:
            return o

    from concourse.dve_spec import Spec, Src0, Src1
    from concourse.dve_uop import (
        DveOpSpec, UopConfig, UopDpConfig, AluOp, AluInp, InpSel, OutSel,
        OutPath, Trigger, DelayInp, ENABLE, N_STAGES,
    )

    def ref(in0, in1, c0, c1, c2):
        v = np.asarray(in0, np.float32)   # src0 = v (PSUM)
        z = np.asarray(in1, np.float32)   # src1 = z (SBUF, interleaved)
        if v.ndim == 2:   # [P, 2T] memory-interleaved stream
            v = v.reshape(v.shape[0], -1, 2)
        if z.ndim == 2:
            z = z.reshape(z.shape[0], -1, 2)
        P, T, _ = z.shape

        def col(c):
            a = np.broadcast_to(np.asarray(c, np.float32), (P, 1))
            return np.array(a[:, 0], np.float32)

        st = [col(c0), col(c1)]
        out = np.empty_like(z)
        for t in range(T):
            for m in range(2):
                st[m] = (1.0 - z[:, t, m]) * st[m] + z[:, t, m] * v[:, t, m]
                out[:, t, m] = st[m]
        return out

    spec = Spec(body=Src0 * Src1, reference=ref)  # stand-in body;
    # compile() below supplies the hand-written uOp program instead of lower().

    def build_uops(ver):
        n_stages = N_STAGES[ver]

        def mk():
            u = UopConfig()
            u.datapath_config = [UopDpConfig() for _ in range(n_stages)]
            return u

        def steady(mult_stage, add_stage):
            U = mk()
            U.enable_input(InpSel.SRC_0, 1)       # chain0 = v (PSUM)
            U.enable_input(InpSel.SRC_1, 2)       # chain1 = z (SBUF)
            U.enable_input(InpSel.ONE_F32, 3)     # chain2 = 1.0
            U.require_inp0 = ENABLE
            U.require_inp1 = ENABLE
            # s0: u = 1 - z
            U.datapath_config[0].enable_alu(
                AluOp.SUBTRACT, AluInp.PREV_DELAY_2, AluInp.PREV_DELAY_1
            ).pass_through_delay(0, 1)
            # s1: m = z*v ; chain2 <- u
            U.datapath_config[1].enable_alu(
                AluOp.MULTIPLY, AluInp.PREV_DELAY_0, AluInp.PREV_DELAY_1
            ).enable_delay_from_src(DelayInp.PREV_ALU_OUT, 2)
            # s2: chain3 <- m ; carry chain2 (u)
            U.datapath_config[2].pass_through_alu().pass_through_delay(2)
            U.datapath_config[2].enable_delay_from_src(DelayInp.PREV_ALU_OUT, 3)
            for k in range(3, mult_stage):
                U.datapath_config[k].pass_through_alu().pass_through_delay(2, 3)
            # mult_stage: p = state * u  (state = A-flop of mult_stage+1)
            U.datapath_config[mult_stage].enable_alu(
                AluOp.MULTIPLY, AluInp.NEXT_ALU_OUT_A, AluInp.PREV_DELAY_2
            ).pass_through_delay(3)
            # add_stage: c' = p + m ; state <- c' (own A-flop)
            blk = U.datapath_config[add_stage]
            blk.enable_alu(AluOp.ADD, AluInp.PREV_ALU_OUT, AluInp.PREV_DELAY_3)
            blk.alu_out_a_enable = ENABLE
            for k in range(add_stage + 1, n_stages):
                U.datapath_config[k].pass_through_alu()
            U.enable_output(OutSel.ALU_OUT, OutPath.WR0_LO)
            U.repeat_count = 1
            return U

        # uop 0 -- seed: c0 -> s4.A, c1 -> s5.A. No consume, no output.
        seed = mk()
        seed.enable_input(InpSel.CONST_0, 1)   # chain0 = c0 (s0 scalar)
        seed.enable_input(InpSel.CONST_1, 2)   # chain1 = c1 (s1 scalar)
        for k in range(4):
            seed.datapath_config[k].pass_through_alu().pass_through_delay(0, 1)
        seed.datapath_config[4].enable_alu(
            AluOp.BYPASS, AluInp.PREV_DELAY_0
        ).pass_through_delay(1)
        seed.datapath_config[4].alu_out_a_enable = ENABLE
        seed.datapath_config[5].enable_alu(AluOp.BYPASS, AluInp.PREV_DELAY_1)
        seed.datapath_config[5].alu_out_a_enable = ENABLE
        for k in range(6, n_stages):
            seed.datapath_config[k].pass_through_alu()
        seed.repeat_count = 2
        seed.trigger = (Trigger.COUNT, Trigger.NONE, Trigger.NONE)
        seed.next_uop = (1, 0, 0)

        H0 = steady(3, 4)
        H0.trigger = (Trigger.COUNT, Trigger.SRC_TENSOR_DONE, Trigger.NONE)
        H0.next_uop = (2, 0, 0)
        H1 = steady(4, 5)
        # stream length is even (2 per timestep): exhaustion fires on H1.
        H1.trigger = (Trigger.SRC_TENSOR_DONE, Trigger.COUNT, Trigger.NONE)
        H1.next_uop = (0, 1, 0)

        for u in (seed, H0, H1):
            u.validate(ver)
        return [seed, H0, H1]

    class HandDveOp(dve_ops.DveOp):
        def compile(self, ver):
            key = (self.name, ver)
            cache = dve_ops._COMPILE_CACHE
            if key not in cache:
                cache[key] = DveOpSpec(
                    name=self.name,
                    opcode=dve_ops.get_dve_sub_opcode(self.name),
                    uops=build_uops(ver),
                    rd1_en=True,
                )
            return cache[key]

    op = HandDveOp(name=OP3_NAME, spec=spec, subdim=False, uops_sha={})
    dve_ops.OPS.append(op)
    dve_ops.CUSTOM_DVE_SPECS[OP3_NAME] = spec
    dve_ops._SUB_OPCODE_FOR_NAME[OP3_NAME] = (
        dve_ops._CUSTOM_DVE_ROW_BASE + len(dve_ops.OPS) - 1
    )
    assert dve_ops._SUB_OPCODE_FOR_NAME[OP3_NAME] < 0x20
    return op


def _csizes(seq_len, chunk):
    assert seq_len % chunk == 0
    n = seq_len // chunk
    if n >= 4 and chunk % 8 == 0:
        # taper: small first/last chunks shorten pipeline fill and drain
        c8 = chunk // 8
        return ([2 * c8, 6 * c8] + [chunk] * (n - 2)
                + [6 * c8, 2 * c8])
    return [chunk] * n


def _segs(cs):
    """Split a chunk into scan segments: [(offset, len)].

    A segment's vh tile is [128, 2*sl] with half m at cols [m*sl:(m+1)*sl];
    matmuls must not cross a PSUM bank (512 fp32), so sl must be exactly
    SEG (halves = whole banks) or <= SEG//2 (both halves in one bank)."""
    out = []
    off = 0
    while off < cs:
        rem = cs - off
        sl = SEG if rem >= SEG else (SEG // 2 if rem > SEG // 2 else rem)
        out.append((off, sl))
        off += sl
    return out


def _build(seq_len, chunk):
    """Build + compile the single-core SPMD Bass program."""
    import concourse.bacc as bacc
    import concourse.tile as tile
    import concourse.mybir as mybir

    dt = mybir.dt
    f32 = dt.float32
    bf16 = dt.bfloat16
    AF = mybir.ActivationFunctionType

    op3 = _register_mingru3_op()

    csizes = _csizes(seq_len, chunk)
    assert sum(csizes) == seq_len

    nc = bacc.Bacc("TRN2", target_bir_lowering=False, debug=False)

    # per-chunk slabs: chunk c occupies cols [2*c_start, 2*c_start+2*cs),
    # within it [k*cs + s] -> x[c_start+s, k*128+p]
    xT_d = nc.dram_tensor("xT", [128, 2 * seq_len], bf16,
                          kind="ExternalInput").ap()
    # packed weights: [wz k0 | wz k1 | wh k0 | wh k1], each [128, H]
    wall_d = nc.dram_tensor("wall", [128, 4 * H], bf16,
                            kind="ExternalInput").ap()
    # packed per-partition columns: [bz_m0, bz_m1, c0_m0, c0_m1]
    cols_d = nc.dram_tensor("cols", [128, 4], f32, kind="ExternalInput").ap()
    # interleaved bf16 output: out[p, 2*T + m] = c[T, m*128+p]
    out_d = nc.dram_tensor("out", [128, 2 * seq_len], bf16,
                           kind="ExternalOutput").ap()

    with tile.TileContext(nc) as tc, ExitStack() as ctx:
        const = ctx.enter_context(tc.tile_pool(name="const", bufs=1))
        xin = ctx.enter_context(tc.tile_pool(name="xin", bufs=6))
        zp = ctx.enter_context(tc.tile_pool(name="z", bufs=4))
        cp = ctx.enter_context(tc.tile_pool(name="c", bufs=4))
        carp = ctx.enter_context(tc.tile_pool(name="car", bufs=2))
        vzp = ctx.enter_context(tc.tile_pool(name="vz", bufs=4, space="PSUM"))
        vhp = ctx.enter_context(tc.tile_pool(name="vh", bufs=2, space="PSUM"))

        # weights + cols ride the Scalar engine's HWDGE queue so their
        # triggers run in parallel with the Sync queue's x-chunk triggers
        # (each trigger costs ~650ns of serial sequencer time).
        wall = const.tile([128, 4 * H], bf16, tag="wall", name="wall")
        nc.scalar.dma_start(wall[:, :2 * H], wall_d[:, :2 * H])   # wz first
        nc.scalar.dma_start(wall[:, 2 * H:], wall_d[:, 2 * H:])   # wh
        cols = const.tile([128, 4], f32, tag="cols", name="cols")
        nc.scalar.dma_start(cols[:], cols_d[:, :])

        # dummy sigmoid loads the ACT table at t~0, overlapped with the
        # wall/x0 DMAs instead of stalling the first real sigmoid.
        dum = const.tile([128, 1], f32, tag="dum", name="dum")
        nc.gpsimd.memset(dum[:], 0.0)
        dumz = const.tile([128, 1], bf16, tag="dumz", name="dumz")
        nc.scalar.activation(dumz[:], dum[:], AF.Sigmoid, scale=1.0)

        # PE warm-up: HAM boots throttled (K=4/8) and only releases after
        # sustained matmul activity. A burst of dummy matmuls during the
        # preamble (inputs are a DVE-memset tile, output a scratch vz slot)
        # brings the PE to full rate before the first real matmul.
        dumb = const.tile([128, 512], bf16, tag="dumb", name="dumb")
        nc.vector.memset(dumb[:], 0.0)
        warm = vzp.tile([128, 512], f32, tag="vz", name="warm")
        for _ in range(16):
            nc.tensor.matmul(warm[:, 0:128], dumb[:, 0:128], dumb[:, 0:128],
                             start=True, stop=True)

        # lhsT slice for matrix j (0=z, 1=h), k-half k, output half m
        def wsl(j, k, m):
            o = j * 2 * H + k * H + m * 128
            return wall[:, o:o + 128]

        # hoist ALL x-DMA triggers ahead of the out-DMAs: the Sync queue is
        # strict FIFO, so an x(c+1) trigger emitted after out(c) would sit
        # behind out(c)'s semaphore wait and cap prefetch at the scan pace
        xts = []
        cb = 0
        for c, cs in enumerate(csizes):
            xt = xin.tile([128, 2 * cs], bf16, tag="xt", name="xt")
            nc.sync.dma_start(xt[:], xT_d[:, 2 * cb:2 * cb + 2 * cs])
            xts.append(xt)
            cb += cs

        co_prev = None
        pcs = 0
        c_start = 0
        for c, cs in enumerate(csizes):
            segs = _segs(cs)
            xt = xts[c]
            co = cp.tile([128, 2 * cs], bf16, tag="c", name="co")

            for sg, (off, sl) in enumerate(segs):
                # ---- PE: 1-bank vz tiles (bufs=4) + one 2-bank vh tile
                # (both halves contiguous; every matmul inside one bank)
                vzs = []
                for m in range(2):
                    vz = vzp.tile([128, sl], f32, tag="vz", name=f"vz{m}")
                    for k in range(2):
                        nc.tensor.matmul(
                            vz[:],
                            wsl(0, k, m),
                            xt[:, k * cs + off:k * cs + off + sl],
                            start=(k == 0), stop=(k == 1),
                        )
                    vzs.append(vz)
                vh = vhp.tile([128, 2 * sl], f32, tag="vh", name=f"vh{sg}")
                for m in range(2):
                    for k in range(2):
                        nc.tensor.matmul(
                            vh[:, m * sl:(m + 1) * sl],
                            wsl(1, k, m),
                            xt[:, k * cs + off:k * cs + off + sl],
                            start=(k == 0), stop=(k == 1),
                        )

                # ---- ACT: z = sigmoid(vz + bz), column-INTERLEAVED
                # (col 2s+m) so the scan's z stream (in1) is a plain 2D AP —
                # the 3D/STT src1 encoding can't carry an s1 scalar AP.
                z = zp.tile([128, 2 * sl], bf16, tag="z", name="z")
                zi = z[:].rearrange("p (s m) -> p m s", m=2)
                for m in range(2):
                    nc.scalar.activation(zi[:, m:m + 1, :], vzs[m][:],
                                         AF.Sigmoid, bias=cols[:, m:m + 1],
                                         scale=1.0)

                # ---- DVE: carry cast + interleaved scan ----
                if c == 0 and sg == 0:
                    s0, s1 = cols[:, 2:3], cols[:, 3:4]
                else:
                    sc = (co_prev[:, 2 * pcs - 2:2 * pcs] if sg == 0
                          else co[:, 2 * off - 2:2 * off])
                    car = carp.tile([128, 2], f32, tag="car", name="car")
                    nc.vector.tensor_copy(car[:], sc)
                    s0, s1 = car[:, 0:1], car[:, 1:2]
                nc.vector._custom_dve(
                    op3,
                    out=co[:, 2 * off:2 * (off + sl)].rearrange(
                        "p (s m) -> p s m", m=2),
                    in0=vh[:].rearrange("p (m s) -> p s m", m=2),
                    in1=z[:],
                    s0=s0, s1=s1)

            nc.sync.dma_start(out_d[:, 2 * c_start:2 * c_start + 2 * cs],
                              co[:])
            co_prev = co
            pcs = cs
            c_start += cs

    nc.compile()
    return nc


def _get(seq_len, chunk):
    key = (seq_len, chunk)
    if key not in _CACHE:
        _CACHE[key] = _build(seq_len, chunk)
    return _CACHE[key]


def _make_in_maps(x, h0, w_h_w, w_h_b, w_z_w, w_z_b, n_cores=N_CORES,
                  seq_len=S, chunk=1024):
    import ml_dtypes
    bf16 = ml_dtypes.bfloat16
    csizes = _csizes(seq_len, chunk)
    wzT = np.asarray(w_z_w, np.float32).T.astype(bf16)   # [D, H]
    whT = np.asarray(w_h_w, np.float32).T.astype(bf16)
    wall = np.concatenate([wzT[:128], wzT[128:], whT[:128], whT[128:]],
                          axis=1)
    bz = np.asarray(w_z_b, np.float32).reshape(2, 128)
    bh = np.asarray(w_h_b, np.float32)
    in_maps = []
    for i in range(n_cores):
        c0 = (np.asarray(h0[i, 0], np.float32) - bh).reshape(2, 128)
        # [128, 4] = [bz_m0, bz_m1, c0_m0, c0_m1]
        cols = np.stack([bz[0], bz[1], c0[0], c0[1]], axis=1)
        xb = np.asarray(x[i], np.float32).astype(bf16)   # [S, 256]
        slabs = []
        cb = 0
        for cs in csizes:
            xc = xb[cb:cb + cs].T.reshape(2, 128, cs)    # [k, p, s]
            slabs.append(np.transpose(xc, (1, 0, 2)).reshape(128, 2 * cs))
            cb += cs
        xT = np.concatenate(slabs, axis=1)               # [128, 2*S]
        in_maps.append({
            "xT": np.ascontiguousarray(xT),
            "wall": np.ascontiguousarray(wall),
            "cols": np.ascontiguousarray(cols),
        })
    return in_maps


def _decode_out(raw, w_h_b, seq_len=S):
    """raw [128, 2*seq_len] bf16 interleaved -> [seq_len, 256] fp32 (+bh)."""
    bh = np.asarray(w_h_b, np.float32)
    a = np.asarray(raw).astype(np.float32).reshape(128, seq_len, 2)
    return np.transpose(a, (1, 2, 0)).reshape(seq_len, 256) + bh


def kernel(x, h0, w_h_w, w_h_b, w_z_w, w_z_b):
    from concourse.bass_utils import run_bass_kernel_spmd

    nc = _get(S, 1024)
    in_maps = _make_in_maps(x, h0, w_h_w, w_h_b, w_z_w, w_z_b)
    res = run_bass_kernel_spmd(nc, in_maps, list(range(N_CORES)))
    out = np.stack(
        [_decode_out(res.results[i]["out"], w_h_b) for i in range(N_CORES)],
        axis=0)
    return out.astype(np.float32)
